# revision 1
# baseline (speedup 1.0000x reference)
"""BitLinear (RMSNorm + per-token int8 act fake-quant + ternary weight fake-quant
+ linear) Trainium2 Bass kernel, data-parallel over 8 NeuronCores.

Strategy
--------
Tokens (B*S = 32768) are sharded 8 ways (4096 tokens/core); W is replicated.
Host prep: ternary weight quantization (per-tensor stat, fp32 semantics
matching the reference) ships as fp8e4 DUPLICATED pairs (-16*w, w) in k-major
layout; x ships as bf16 (halves input DMA; ~0.3% effect well inside the 2e-2
gate); the output returns as bf16 and is upcast on host (same rationale).

The matmul runs in fp8 DoubleRow mode (2 MACs/cell/cycle) while staying
integer-exact via a hi/lo split of the int8 activations:
    q = round(xn*a) in [-127,127];  hneg = -round(q/16);  l = q + 16*hneg
    q*w = hneg*(-16w) + l*w         (all of hneg, l, -16w, w fp8e4-exact)
Each DoubleRow matmul contracts the (hi,lo) pair per k: per 128-token tile,
16 k-blocks x 4 out-groups x 512 cols at 0.5 cycles/col = 16384 PE cycles,
half of the bf16 rate. Verified bit-exact on hardware (mm_test.py).

Work is issued in SUPER-TILES of 256 tokens (2 PE tiles) to halve the DMA
count and semaphore traffic per token — the SP sequencer (which issues
x-loads and q-transposes and serializes on each DMA's waits) was the
measured cadence limiter at 1 tile per issue group. Engine budget per
super-tile (PE = 13.66us):
  ACT  ~12.1us: Square+accum x2, sqrt, u = x*s + C (magic RNE round) x2,
                psum evacuation * s3 -> bf16 x2 (deferred one super-tile so
                its matmul-wait never clogs the ACT queue), out-DMA issue
  DVE  ~12.4us: absmax (one [128,2,2048] reduce), per-token scalar chain,
                q = u - C -> bf16, t2 = qt*(-1/16) - 192 in ONE 2x-mode op
                (bf16 magic: ulp=1 in [128,256)) written in place over q's
                dead storage, l = (hneg*16) + qt -> fp8 per kt-half
  Pool  ~6.5us: hneg = t2 + 192 -> fp8 per kt-half (4 ops; the kt-half
                granularity lets each 8-kt matmul group start while the
                next half's hi/lo pair is still being built, keeping the
                PE streak alive across tile boundaries)
  DMA  ~10.9us: x in (prefetched 1 super-tile ahead), q transpose (xbar),
                out; all 16 weight chunks issue around super-tile 0 so every
                matmul's RAW dependency on them is tracked.

Timeline-sim: 311.6us/core (vs 476.7us bf16 baseline). The gap to the
218.5us PE roofline is pipeline-fill (~30us), the tile framework's
vector-clock semaphore-recycle stalls (~30us, all engine sequencers park on
EventSemaphore waits every few super-tiles — the sem window also caps PE
runahead, so deeper buffering cannot hide them), and the PE pstate ramp
after each streak reset (~30us). Next lever if revisited: For_i_pipelined
hardware loops with staggered_reset to stop burning semaphores per
super-tile. NOTE: slicing a rearranged AP view (view[:, h, ks]) silently
mis-strides on hardware while passing the no-exec sim — slice the
underlying tile first, then rearrange.
"""
import numpy as np
from contextlib import ExitStack

import concourse.bacc as bacc
import concourse.tile as tile
from concourse import mybir
from concourse.bass_utils import run_bass_kernel_spmd

F32 = mybir.dt.float32
BF16 = mybir.dt.bfloat16
FP8 = mybir.dt.float8e4
AL = mybir.AluOpType
AF = mybir.ActivationFunctionType
AX = mybir.AxisListType
PM = mybir.MatmulPerfMode.DoubleRow

B, S, DIN, DOUT = 4, 8192, 2048, 2048
NCORES = 8
TOK = B * S                  # 32768
TPC = TOK // NCORES          # 4096 tokens per core
NST = TPC // 256             # 16 super-tiles (2 PE tiles each) per core
KB = DIN // 128              # 16 contraction blocks per token tile
OGW = 512                    # psum free dim per matmul
OG = DOUT // OGW             # 4 output groups

C_MAGIC = 12582912.0         # 1.5 * 2^23: fp32 +C/-C rounds to nearest int (RNE)
C_BF16 = 192.0               # bf16 magic for h: ulp=1 in [128,256)

_CACHE = {}


def _build(cfg=None):
    cfg = {**dict(xp=3, up=3, qp=2, qtp=2, hlp=2, op=2), **(cfg or {})}
    nc = bacc.Bacc("TRN2", target_bir_lowering=False, debug=False,
                   num_devices=NCORES)
    x_d = nc.declare_dram_parameter("x", [TPC, DIN], BF16, isOutput=False)
    w2_d = nc.declare_dram_parameter("w2", [DIN, 2 * DOUT], FP8, isOutput=False)
    sc_d = nc.declare_dram_parameter("sc", [1, 1], F32, isOutput=False)
    o_d = nc.declare_dram_parameter("out", [TPC, DOUT], BF16, isOutput=True)

    with tile.TileContext(nc) as tc:
        with ExitStack() as ctx:
            cst = ctx.enter_context(tc.tile_pool(name="cst", bufs=1))
            w2p = ctx.enter_context(tc.tile_pool(name="w2p", bufs=1))
            xp = ctx.enter_context(tc.tile_pool(name="xp", bufs=cfg["xp"]))
            up = ctx.enter_context(tc.tile_pool(name="up", bufs=cfg["up"]))
            qp = ctx.enter_context(tc.tile_pool(name="qp", bufs=cfg["qp"]))
            qtp = ctx.enter_context(tc.tile_pool(name="qtp", bufs=cfg["qtp"]))
            hlp = ctx.enter_context(tc.tile_pool(name="hlp", bufs=cfg["hlp"]))
            op = ctx.enter_context(tc.tile_pool(name="op", bufs=cfg["op"]))
            st = ctx.enter_context(tc.tile_pool(name="st", bufs=6))
            pso = ctx.enter_context(tc.tile_pool(name="pso", bufs=1, space="PSUM"))

            # ---- constants; first x super-tile ahead of the weight chunks
            xpre = xp.tile([128, 2, DIN], BF16, name="xt", tag="xtile")
            nc.sync.dma_start(
                out=xpre,
                in_=x_d[0:256, :].rearrange("(h p) k -> p h k", p=128))
            scb = cst.tile([128, 1], F32, name="scb")
            nc.sync.dma_start(out=scb, in_=sc_d[:].to_broadcast((128, 1)))
            inv127 = scb[:, 0:1]   # 1/(127*w_scale)
            cmag = cst.tile([128, 1], F32, name="cmag")
            nc.vector.memset(cmag, C_MAGIC)
            ceps = cst.tile([128, 1], F32, name="ceps")
            nc.vector.memset(ceps, 1e-6)
            warmt = cst.tile([128, 1], F32, name="warmt")
            nc.scalar.activation(out=warmt, in_=cmag, func=AF.Square)
            nc.scalar.activation(out=warmt, in_=cmag, func=AF.Sqrt)

            # ---- weight pairs (-16w, w), fp8 k-major, [128, kt, 2, DOUT]
            w2 = w2p.tile([128, KB, 2, DOUT], FP8, name="w2")

            # first half of the weight chunks up front (they fill the DMA
            # device while super-tile 0's front-end computes); the second
            # half issues inside super-tile 0 after its transposes so the
            # first transpose is never queued behind them. All 16 chunks are
            # emitted before any matmul so the RAW dependency is tracked.
            for kt in range(8):
                nc.sync.dma_start(out=w2[:, kt, :, :],
                                  in_=w2_d[kt * 128:(kt + 1) * 128, :])

            # ---- super-tiles (2 token tiles each) ----
            pend = []

            def _flush_epilogue(e):
                jt, pj, otj, s3j = e
                for h in range(2):
                    nc.scalar.mul(out=otj[:, h, :], in_=pj[h],
                                  mul=s3j[:, h:h + 1])
                nc.scalar.dma_start(
                    out=o_d[jt * 256:(jt + 1) * 256, :]
                    .rearrange("(h p) k -> p h k", p=128),
                    in_=otj)

            xtiles = [xpre]
            for it in range(NST):
                xt = xtiles[it]
                # prefetch the next super-tile's x ahead of this one's
                # transposes so a transpose's q-wait never delays the load
                if it + 1 < NST:
                    xn = xp.tile([128, 2, DIN], BF16, name="xt", tag="xtile")
                    nc.sync.dma_start(
                        out=xn,
                        in_=x_d[(it + 1) * 256:(it + 2) * 256, :]
                        .rearrange("(h p) k -> p h k", p=128))
                    xtiles.append(xn)

                # per-token stats on raw x: sumsq (ACT) and absmax (DVE)
                scr = up.tile([128, 2, DIN], F32, name="scr")
                ss = st.tile([128, 2], F32, name="ss")
                for h in range(2):
                    nc.scalar.activation(out=scr[:, h, :], in_=xt[:, h, :],
                                         func=AF.Square,
                                         accum_out=ss[:, h:h + 1])
                mx = st.tile([128, 2], F32, name="mx")
                nc.vector.tensor_reduce(out=mx, in_=xt, axis=AX.X, op=AL.max,
                                        apply_absolute_value=True)
                # rms = 1/sqrt(ss/DIN + 1e-6)
                sqv = st.tile([128, 2], F32, name="sqv")
                nc.scalar.activation(out=sqv, in_=ss, func=AF.Sqrt, bias=ceps,
                                     scale=1.0 / DIN)
                rms = st.tile([128, 2], F32, name="rms")
                nc.vector.reciprocal(out=rms, in_=sqv)
                # d = (mx + 1e-5)*rms  (~= rms*mx + 1e-5: the eps only
                # guards div-by-zero; the difference is ~1e-5*(rms-1), a
                # ~3e-6 relative perturbation of the quant scale)
                d = st.tile([128, 2], F32, name="d")
                nc.vector.scalar_tensor_tensor(out=d, in0=mx, scalar=1e-5,
                                               in1=rms, op0=AL.add,
                                               op1=AL.mult)
                rcd = st.tile([128, 2], F32, name="rcd")
                nc.vector.reciprocal(out=rcd, in_=d)
                s = st.tile([128, 2], F32, name="s")
                nc.vector.scalar_tensor_tensor(out=s, in0=rcd, scalar=127.0,
                                               in1=rms, op0=AL.mult,
                                               op1=AL.mult)
                s3 = st.tile([128, 2], F32, name="s3")
                nc.vector.tensor_scalar(out=s3, in0=d, scalar1=inv127,
                                        scalar2=None, op0=AL.mult)

                # q = RNE(x*s) via fp32 magic (ACT fma per half + DVE unbias)
                q = qp.tile([128, 2, DIN], BF16, name="q")
                qt = qtp.tile([128, 2, KB, 128], BF16, name="qt")
                if it == 0:
                    # fine-grained first super-tile: tile A's transpose and
                    # hi/lo chain start without waiting for tile B's front end
                    for h in range(2):
                        nc.scalar.activation(out=scr[:, h, :], in_=xt[:, h, :],
                                             func=AF.Identity, bias=cmag,
                                             scale=s[:, h:h + 1])
                        nc.vector.tensor_scalar(out=q[:, h, :],
                                                in0=scr[:, h, :],
                                                scalar1=C_MAGIC, scalar2=None,
                                                op0=AL.subtract)
                        nc.sync.dma_start_transpose(qt[:, h], q[:, h, :])
                    for kt in range(8, KB):
                        nc.sync.dma_start(out=w2[:, kt, :, :],
                                          in_=w2_d[kt * 128:(kt + 1) * 128, :])
                else:
                    for h in range(2):
                        nc.scalar.activation(out=scr[:, h, :], in_=xt[:, h, :],
                                             func=AF.Identity, bias=cmag,
                                             scale=s[:, h:h + 1])
                    nc.vector.tensor_scalar(out=q, in0=scr, scalar1=C_MAGIC,
                                            scalar2=None, op0=AL.subtract)
                    # one xbar transpose for both token tiles
                    nc.sync.dma_start_transpose(
                        qt, q.rearrange("p h k -> p (h k)"))

                # t2 overwrites q's storage in place (q is dead once the
                # transpose has read it; the WAR dep serializes them)
                t2 = q.rearrange("p h (a b) -> p h a b", a=KB)
                qhl = hlp.tile([128, 2, KB, 2, 128], FP8, name="qhl")
                ot = op.tile([128, 2, DOUT], BF16, name="ot")
                # t2 in one 2x-mode DVE op (bf16 in/out) — shorter chain
                # than two serial Pool halves; Pool keeps only hneg
                nc.vector.tensor_scalar(out=t2, in0=qt, scalar1=-1.0 / 16.0,
                                        scalar2=-C_BF16, op0=AL.mult,
                                        op1=AL.add)
                poss = []
                HK = KB // 2
                for h in range(2):
                    pos = pso.tile([128, DOUT], F32, name=f"pos{h}",
                                   tag=f"pos{h}")
                    poss.append(pos)
                    for j in range(2):
                        ks = slice(j * HK, (j + 1) * HK)
                        # slice q first, then view as kt-blocks — slicing a
                        # rearranged view mis-strides on the DMA/engine APs
                        t2hj = q[:, h, j * HK * 128:(j + 1) * HK * 128] \
                            .rearrange("p (a b) -> p a b", a=HK)
                        nc.gpsimd.tensor_scalar(out=qhl[:, h, ks, 0, :],
                                                in0=t2hj,
                                                scalar1=C_BF16, scalar2=None,
                                                op0=AL.add)
                        nc.vector.scalar_tensor_tensor(
                            out=qhl[:, h, ks, 1, :],
                            in0=qhl[:, h, ks, 0, :],
                            scalar=16.0, in1=qt[:, h, ks],
                            op0=AL.mult, op1=AL.add)
                        for kt in range(j * HK, (j + 1) * HK):
                            lhsT = qhl[:, h, kt, :, :]
                            for og in range(OG):
                                nc.tensor.matmul(
                                    pos[:, og * OGW:(og + 1) * OGW],
                                    lhsT=lhsT,
                                    rhs=w2[:, kt, :, og * OGW:(og + 1) * OGW],
                                    start=(kt == 0), stop=(kt == KB - 1),
                                    perf_mode=PM)
                # the PREVIOUS super-tile's psum evacuation + writeback are
                # emitted at the very end of this body so their matmul-
                # completion waits never sit ahead of front-end work
                if pend:
                    _flush_epilogue(pend.pop(0))
                pend.append((it, poss, ot, s3))
                if it == NST - 1:
                    _flush_epilogue(pend.pop())

    nc.compile()
    return nc


def kernel(x, gamma, W):
    import ml_dtypes

    x = np.asarray(x, dtype=np.float32)
    gamma = np.asarray(gamma, dtype=np.float32)
    W = np.asarray(W, dtype=np.float32)

    # host prep: ternary weight pairs + the global scale, fp32 semantics
    # matching the reference: w_scale = 1/(mean|W| + 1e-5)
    m = np.float32(np.abs(W).astype(np.float64).mean())
    denom = np.float32(m + np.float32(1e-5))
    ws = np.float32(np.float32(1.0) / denom)
    wqh = np.clip(np.rint((W * ws).astype(np.float32)), -1.0, 1.0)
    wt = np.ascontiguousarray(wqh.T)                      # [DIN, DOUT]
    w2 = np.empty((DIN, 2, DOUT), dtype=ml_dtypes.float8_e4m3)
    w2[:, 0, :] = (-16.0 * wt).astype(ml_dtypes.float8_e4m3)
    w2[:, 1, :] = wt.astype(ml_dtypes.float8_e4m3)
    w2 = w2.reshape(DIN, 2 * DOUT)
    sc = np.array([[1.0 / (127.0 * float(ws))]], dtype=np.float32)

    if not np.all(gamma == 1.0):
        x = x * gamma  # reference order is (x*rms)*gamma; ~1ulp difference
    xb = x.reshape(TOK, DIN).astype(ml_dtypes.bfloat16)

    if "nc" not in _CACHE:
        _CACHE["nc"] = _build()
    nc = _CACHE["nc"]

    in_maps = [
        {"x": xb[c * TPC:(c + 1) * TPC], "w2": w2, "sc": sc}
        for c in range(NCORES)
    ]
    res = run_bass_kernel_spmd(nc, in_maps, list(range(NCORES)))
    out = np.concatenate([res.results[c]["out"] for c in range(NCORES)],
                         axis=0)
    return out.astype(np.float32).reshape(B, S, DOUT)


if __name__ == "__main__":
    rng = np.random.default_rng(0)
    x = rng.standard_normal((B, S, DIN), dtype=np.float32)
    gamma = np.ones((DIN,), dtype=np.float32)
    bound = 1.0 / np.sqrt(DIN)
    W = rng.uniform(-bound, bound, (DOUT, DIN)).astype(np.float32)
    out = kernel(x, gamma, W)
    print("out", out.shape, out.dtype, float(np.abs(out).mean()))



# revision 2
# speedup vs baseline: 1.1344x; 1.1344x over previous
"""BitLinear Trainium2 Bass kernel — data-parallel over 8 NeuronCores.

v1 structure (coarse per-super-tile ops, deferred epilogue) with three
rebalances that cut the DVE critical chain and spread load:

  q = round(xn*a) in [-127,127];  qs = q + 8 in [-119,135] (bf16-exact)
  h = floor(qs/16) in [-8,8];     l = (qs mod 16) - 8 in [-8,7]
  q*w = (16h)*w + l*w             (16h in [-128,128] and l fp8e4-exact)

  * matmul pair is (16h, l) against a stride-0 BROADCAST rhs (w, w), so W
    ships and resides once (32KB/partition instead of 64KB for the
    (-16w, w) pair layout) and the freed SBUF buys deeper pipeline pools.
  * l via ONE 2x-mode DVE tensor_scalar (op0=mod 16, op1=sub 8) straight
    from the transposed qs — replaces the 1x scalar_tensor_tensor chain
    that ran after the h build (saves ~2.3us DVE/ST + shortens the chain).
  * h via bf16 magic: t2 = qs*(-1/16) - 191.53125 rounds (bf16 ulp=1 in
    [184,200]) to -floor(qs/16) - 192, no ties since qs-7.5 is never
    ==8 mod 16; Pool adds 192 -> fp8.
  * absmax as TT(abs_max) fold + reduce (3.3us vs 4.3us 1x reduce).
  * psum evacuation split: h0 on ACT, h1 on Pool.
  * x prefetch DMA issued from the Pool queue so an x-slot wait can never
    head-of-line block the q transposes on SP.
"""
import numpy as np
from contextlib import ExitStack

import concourse.bacc as bacc
import concourse.tile as tile
from concourse import mybir
from concourse.bass_utils import run_bass_kernel_spmd

F32 = mybir.dt.float32
BF16 = mybir.dt.bfloat16
FP8 = mybir.dt.float8e4
AL = mybir.AluOpType
AF = mybir.ActivationFunctionType
AX = mybir.AxisListType
PM = mybir.MatmulPerfMode.DoubleRow

B, S, DIN, DOUT = 4, 8192, 2048, 2048
NCORES = 8
TOK = B * S                  # 32768
TPC = TOK // NCORES          # 4096 tokens per core
NST = TPC // 256             # 16 super-tiles (2 PE tiles each) per core
KB = DIN // 128              # 16 contraction blocks per token tile
OGW = 512                    # psum free dim per matmul
OG = DOUT // OGW             # 4 output groups

C_MAGIC = 12582912.0         # 1.5 * 2^23: fp32 +C rounds to nearest int (RNE)
C_UNB = C_MAGIC              # unbias to q in [-127,127]
# bf16 magic at ulp=16: y = q + 0.5 + 3072 in [2945.5, 3199.5] rounds (bf16
# ulp 16 in [2048,4096), never a tie) to 3072 + 16*h with h = floor((q+8)/16)
T2_BIAS = 3072.5
T2_SUB = 3072.0

_CACHE = {}


def _build(cfg=None):
    cfg = {**dict(xp=4, up=4, qp=2, qtp=2, hlp=2, op=2, tmp=2,
                  evac="act", ub1=2), **(cfg or {})}
    nc = bacc.Bacc("TRN2", target_bir_lowering=False, debug=False,
                   num_devices=NCORES)
    x_d = nc.declare_dram_parameter("x", [TPC, DIN], BF16, isOutput=False)
    w2_d = nc.declare_dram_parameter("w2", [DIN, DOUT], FP8, isOutput=False)
    sc_d = nc.declare_dram_parameter("sc", [1, 1], F32, isOutput=False)
    o_d = nc.declare_dram_parameter("out", [TPC, DOUT], BF16, isOutput=True)

    with tile.TileContext(nc) as tc:
        with ExitStack() as ctx:
            cst = ctx.enter_context(tc.tile_pool(name="cst", bufs=1))
            w2p = ctx.enter_context(tc.tile_pool(name="w2p", bufs=1))
            xp = ctx.enter_context(tc.tile_pool(name="xp", bufs=cfg["xp"]))
            up = ctx.enter_context(tc.tile_pool(name="up", bufs=cfg["up"]))
            qp = ctx.enter_context(tc.tile_pool(name="qp", bufs=cfg["qp"]))
            qtp = ctx.enter_context(tc.tile_pool(name="qtp", bufs=cfg["qtp"]))
            hlp = ctx.enter_context(tc.tile_pool(name="hlp", bufs=cfg["hlp"]))
            op = ctx.enter_context(tc.tile_pool(name="op", bufs=cfg["op"]))
            st = ctx.enter_context(tc.tile_pool(name="st", bufs=6))
            tmp = ctx.enter_context(tc.tile_pool(name="tmp", bufs=cfg["tmp"]))
            pso = ctx.enter_context(tc.tile_pool(name="pso", bufs=1, space="PSUM"))

            # ---- constants; first x super-tile (split per half) first
            xpre = xp.tile([128, 2, DIN], BF16, name="xt", tag="xtile")
            for h in range(2):
                nc.sync.dma_start(
                    out=xpre[:, h, :],
                    in_=x_d[h * 128:(h + 1) * 128, :])
            scb = cst.tile([128, 1], F32, name="scb")
            nc.sync.dma_start(out=scb, in_=sc_d[:].to_broadcast((128, 1)))
            inv127 = scb[:, 0:1]   # 1/(127*w_scale)
            cmag = cst.tile([128, 1], F32, name="cmag")
            nc.vector.memset(cmag, C_MAGIC)
            ceps = cst.tile([128, 1], F32, name="ceps")
            nc.vector.memset(ceps, 1e-6)
            warmt = cst.tile([128, 1], F32, name="warmt")
            nc.scalar.activation(out=warmt, in_=cmag, func=AF.Square)
            nc.scalar.activation(out=warmt, in_=cmag, func=AF.Sqrt)

            # ---- weight pairs (-16w, w), fp8 k-major, [128, kt, 2, DOUT]
            # Only chunks 0-1 load ahead of super-tile 0's q transpose; the
            # rest are emitted inside super-tile 0 (they arrive well before
            # the matmul wave reaches them). x for super-tile 1 also loads
            # up front so its front-end is never starved.
            w2 = w2p.tile([128, KB, DOUT], FP8, name="w2")
            nc.sync.dma_start(out=w2[:, 0, :], in_=w2_d[0:128, :])
            xpre1 = xp.tile([128, 2, DIN], BF16, name="xt", tag="xtile")
            nc.sync.dma_start(
                out=xpre1,
                in_=x_d[256:512, :].rearrange("(h p) k -> p h k", p=128))
            nc.sync.dma_start(out=w2[:, 1, :], in_=w2_d[128:256, :])

            def wrhs(kt, og):
                return w2[:, kt, og * OGW:(og + 1) * OGW] \
                    .unsqueeze(1).to_broadcast((128, 2, OGW))

            pend = []

            def _flush_epilogue(e):
                # GPSIMD cannot access PSUM on HW: evac h0 on ACT, h1 on DVE
                jt, pj, otj, s3j = e
                for h in range(2):
                    for g in range(2):
                        dst = otj[:, h, g * 1024:(g + 1) * 1024]
                        if cfg["evac"] == "split" and h == 1:
                            nc.vector.tensor_scalar(out=dst, in0=pj[2 * h + g],
                                                    scalar1=s3j[:, h:h + 1],
                                                    scalar2=None, op0=AL.mult)
                        else:
                            nc.scalar.mul(out=dst, in_=pj[2 * h + g],
                                          mul=s3j[:, h:h + 1])
                nc.sync.dma_start(
                    out=o_d[jt * 256:(jt + 1) * 256, :]
                    .rearrange("(h p) k -> p h k", p=128),
                    in_=otj)

            xtiles = [xpre, xpre1]
            for it in range(NST):
                xt = xtiles[it]
                q = qp.tile([128, 2, DIN], BF16, name="q")
                qt = qtp.tile([128, 2, KB, 128], BF16, name="qt")
                qhl = hlp.tile([128, 2, KB, 2, 128], FP8, name="qhl")
                ot = op.tile([128, 2, DOUT], BF16, name="ot")
                ss = st.tile([128, 2], F32, name="ss")
                mx = st.tile([128, 2], F32, name="mx")
                HK = KB // 2

                if it == 0:
                    # fine-grained early super-tiles: per-half chain so the
                    # first matmuls fire early and runahead builds fast
                    s3 = st.tile([128, 2], F32, name="s3")
                    poss = []
                    scrs = []
                    for h in range(2):
                        scr_h = up.tile([128, DIN], F32, name="scr")
                        scrs.append(scr_h)
                        nc.scalar.activation(out=scr_h, in_=xt[:, h, :],
                                             func=AF.Square,
                                             accum_out=ss[:, h:h + 1])
                        nc.vector.tensor_reduce(out=mx[:, h:h + 1],
                                                in_=xt[:, h, :],
                                                axis=AX.X, op=AL.max,
                                                apply_absolute_value=True)
                        ts1 = st.tile([128, 1], F32, name="ts1",
                                      tag=f"ts1{h}")
                        nc.vector.tensor_scalar(out=ts1, in0=mx[:, h:h + 1],
                                                scalar1=1e-5,
                                                scalar2=1.0 / 127.0,
                                                op0=AL.add, op1=AL.mult)
                        s = st.tile([128, 1], F32, name="s", tag=f"s{h}")
                        nc.vector.reciprocal(out=s, in_=ts1)
                        sqv = st.tile([128, 1], F32, name="sqv", tag=f"sqv{h}")
                        nc.scalar.activation(out=sqv, in_=ss[:, h:h + 1],
                                             func=AF.Sqrt, bias=ceps,
                                             scale=1.0 / DIN)
                        rms = st.tile([128, 1], F32, name="rms", tag=f"rms{h}")
                        nc.vector.reciprocal(out=rms, in_=sqv)
                        nc.vector.scalar_tensor_tensor(out=s3[:, h:h + 1],
                                                       in0=rms, scalar=127.0,
                                                       in1=ts1, op0=AL.mult,
                                                       op1=AL.mult)
                        nc.vector.tensor_scalar(out=s3[:, h:h + 1],
                                                in0=s3[:, h:h + 1],
                                                scalar1=inv127, scalar2=None,
                                                op0=AL.mult)
                        nc.scalar.activation(out=scr_h, in_=xt[:, h, :],
                                             func=AF.Identity, bias=cmag,
                                             scale=s[:, 0:1])
                        nc.vector.tensor_scalar(out=q[:, h, :],
                                                in0=scr_h,
                                                scalar1=C_UNB, scalar2=None,
                                                op0=AL.subtract)
                        nc.sync.dma_start_transpose(qt[:, h], q[:, h, :])
                        if it == 0 and h == 0:
                            for kt in range(2, KB):
                                nc.sync.dma_start(
                                    out=w2[:, kt, :],
                                    in_=w2_d[kt * 128:(kt + 1) * 128, :])
                        t2 = q[:, h, :].rearrange("p (a b) -> p a b", a=KB)
                        nc.vector.tensor_scalar(out=t2, in0=qt[:, h],
                                                scalar1=T2_BIAS, scalar2=None,
                                                op0=AL.add)
                        pg = [pso.tile([128, 2 * OGW], F32,
                                       name=f"pos{h}{g}", tag=f"pos{h}{g}")
                              for g in range(2)]
                        poss.extend(pg)
                        for j in range(2):
                            ks = slice(j * HK, (j + 1) * HK)
                            t2hj = q[:, h, j * HK * 128:(j + 1) * HK * 128] \
                                .rearrange("p (a b) -> p a b", a=HK)
                            nc.gpsimd.tensor_scalar(out=qhl[:, h, ks, 0, :],
                                                    in0=t2hj, scalar1=T2_SUB,
                                                    scalar2=None,
                                                    op0=AL.subtract)
                            nc.vector.scalar_tensor_tensor(
                                out=qhl[:, h, ks, 1, :],
                                in0=qhl[:, h, ks, 0, :], scalar=-1.0,
                                in1=qt[:, h, ks], op0=AL.mult, op1=AL.add)
                            for kt in range(j * HK, (j + 1) * HK):
                                for og in range(OG):
                                    nc.tensor.matmul(
                                        pg[og // 2][:, (og % 2) * OGW:
                                                    (og % 2 + 1) * OGW],
                                        lhsT=qhl[:, h, kt, :, :],
                                        rhs=wrhs(kt, og),
                                        start=(kt == 0), stop=(kt == KB - 1),
                                        perf_mode=PM)
                    # prefetch x two super-tiles ahead (0 and 1 preloaded)
                    if it + 2 < NST:
                        xn = xp.tile([128, 2, DIN], BF16, name="xt",
                                     tag="xtile")
                        nc.sync.dma_start(
                            out=xn,
                            in_=x_d[(it + 2) * 256:(it + 3) * 256, :]
                            .rearrange("(h p) k -> p h k", p=128))
                        xtiles.append(xn)
                    if pend:
                        _flush_epilogue(pend.pop(0))
                    pend.append((it, poss, ot, s3))
                    continue

                # ---- steady state: coarse per-super-tile ops ----
                scrs = []
                for h in range(2):
                    scr_h = up.tile([128, DIN], F32, name="scr")
                    scrs.append(scr_h)
                    nc.scalar.activation(out=scr_h, in_=xt[:, h, :],
                                         func=AF.Square,
                                         accum_out=ss[:, h:h + 1])
                for h in range(2):
                    nc.vector.tensor_reduce(out=mx[:, h:h + 1],
                                            in_=xt[:, h, :], axis=AX.X,
                                            op=AL.max,
                                            apply_absolute_value=True)
                # quant scale: s = 127/(mx + 1e-5) — the rms cancels out of
                # a_scale*rms, so the sumsq/sqrt path is NOT on the q chain
                ts1 = st.tile([128, 2], F32, name="ts1", tag="ts1")
                nc.vector.tensor_scalar(out=ts1, in0=mx, scalar1=1e-5,
                                        scalar2=1.0 / 127.0, op0=AL.add,
                                        op1=AL.mult)
                s = st.tile([128, 2], F32, name="s2", tag="s2")
                nc.vector.reciprocal(out=s, in_=ts1)
                # output scale s3 = (mx+1e-5)*rms/(127*ws) — off-chain, only
                # needed by the (deferred) psum evacuation
                sqv = st.tile([128, 2], F32, name="sqv2", tag="sqv2")
                nc.scalar.activation(out=sqv, in_=ss, func=AF.Sqrt, bias=ceps,
                                     scale=1.0 / DIN)
                rms = st.tile([128, 2], F32, name="rms2", tag="rms2")
                nc.vector.reciprocal(out=rms, in_=sqv)
                s3 = st.tile([128, 2], F32, name="s3", tag="s3")
                nc.vector.scalar_tensor_tensor(out=s3, in0=rms,
                                               scalar=127.0, in1=ts1,
                                               op0=AL.mult, op1=AL.mult)
                nc.vector.tensor_scalar(out=s3, in0=s3, scalar1=inv127,
                                        scalar2=None, op0=AL.mult)

                # qs = RNE(x*s) + 8 via fp32 magic (per-half scratch)
                for h in range(2):
                    nc.scalar.activation(out=scrs[h], in_=xt[:, h, :],
                                         func=AF.Identity, bias=cmag,
                                         scale=s[:, h:h + 1])
                    ub = cfg.get("ub1")
                    eng = nc.gpsimd if (ub == 2 or (h == 1 and ub)) \
                        else nc.vector
                    eng.tensor_scalar(out=q[:, h, :], in0=scrs[h],
                                      scalar1=C_UNB, scalar2=None,
                                      op0=AL.subtract)
                nc.sync.dma_start_transpose(
                    qt, q.rearrange("p h k -> p (h k)"))
                # prefetch x two super-tiles ahead (xp=4 keeps the slot-free
                # wait at zero; emitted after the transpose so it can never
                # delay it in the queue)
                if it + 2 < NST:
                    xn = xp.tile([128, 2, DIN], BF16, name="xt", tag="xtile")
                    nc.sync.dma_start(
                        out=xn,
                        in_=x_d[(it + 2) * 256:(it + 3) * 256, :]
                        .rearrange("(h p) k -> p h k", p=128))
                    xtiles.append(xn)

                # t2 = qt + 3072.5 -> bf16 = 3072 + 16h, one 4x op (over
                # q's dead storage — WAR after the transpose read)
                t2 = q.rearrange("p h (a b) -> p h a b", a=KB)
                nc.vector.tensor_scalar(out=t2, in0=qt, scalar1=T2_BIAS,
                                        scalar2=None, op0=AL.add)
                poss = []
                for h in range(2):
                    pg = [pso.tile([128, 2 * OGW], F32, name=f"pos{h}{g}",
                                   tag=f"pos{h}{g}") for g in range(2)]
                    poss.extend(pg)
                    for j in range(2):
                        ks = slice(j * HK, (j + 1) * HK)
                        t2hj = q[:, h, j * HK * 128:(j + 1) * HK * 128] \
                            .rearrange("p (a b) -> p a b", a=HK)
                        nc.gpsimd.tensor_scalar(out=qhl[:, h, ks, 0, :],
                                                in0=t2hj, scalar1=T2_SUB,
                                                scalar2=None, op0=AL.subtract)
                        nc.vector.scalar_tensor_tensor(
                            out=qhl[:, h, ks, 1, :],
                            in0=qhl[:, h, ks, 0, :], scalar=-1.0,
                            in1=qt[:, h, ks], op0=AL.mult, op1=AL.add)
                        for kt in range(j * HK, (j + 1) * HK):
                            for og in range(OG):
                                nc.tensor.matmul(
                                    pg[og // 2][:, (og % 2) * OGW:
                                                (og % 2 + 1) * OGW],
                                    lhsT=qhl[:, h, kt, :, :],
                                    rhs=wrhs(kt, og),
                                    start=(kt == 0), stop=(kt == KB - 1),
                                    perf_mode=PM)
                if pend:
                    _flush_epilogue(pend.pop(0))
                pend.append((it, poss, ot, s3))
                if it == NST - 1:
                    jt, pj, otj, s3j = pend.pop()
                    for h in range(2):
                        for g in range(2):
                            nc.scalar.mul(
                                out=otj[:, h, g * 1024:(g + 1) * 1024],
                                in_=pj[2 * h + g], mul=s3j[:, h:h + 1])
                        nc.sync.dma_start(
                            out=o_d[jt * 256 + h * 128:
                                    jt * 256 + (h + 1) * 128, :],
                            in_=otj[:, h, :])

    nc.compile()
    return nc


def kernel(x, gamma, W):
    import ml_dtypes

    x = np.asarray(x, dtype=np.float32)
    gamma = np.asarray(gamma, dtype=np.float32)
    W = np.asarray(W, dtype=np.float32)

    # host prep: ternary weight pairs + the global scale, fp32 semantics
    # matching the reference: w_scale = 1/(mean|W| + 1e-5)
    m = np.float32(np.abs(W).astype(np.float64).mean())
    denom = np.float32(m + np.float32(1e-5))
    ws = np.float32(np.float32(1.0) / denom)
    wqh = np.clip(np.rint((W * ws).astype(np.float32)), -1.0, 1.0)
    w2 = np.ascontiguousarray(wqh.T).astype(ml_dtypes.float8_e4m3)
    sc = np.array([[1.0 / (127.0 * float(ws))]], dtype=np.float32)

    if not np.all(gamma == 1.0):
        x = x * gamma  # reference order is (x*rms)*gamma; ~1ulp difference
    xb = x.reshape(TOK, DIN).astype(ml_dtypes.bfloat16)

    if "nc" not in _CACHE:
        _CACHE["nc"] = _build()
    nc = _CACHE["nc"]

    in_maps = [
        {"x": xb[c * TPC:(c + 1) * TPC], "w2": w2, "sc": sc}
        for c in range(NCORES)
    ]
    res = run_bass_kernel_spmd(nc, in_maps, list(range(NCORES)))
    out = np.concatenate([res.results[c]["out"] for c in range(NCORES)],
                         axis=0)
    return out.astype(np.float32).reshape(B, S, DOUT)


if __name__ == "__main__":
    rng = np.random.default_rng(0)
    x = rng.standard_normal((B, S, DIN), dtype=np.float32)
    gamma = np.ones((DIN,), dtype=np.float32)
    bound = 1.0 / np.sqrt(DIN)
    W = rng.uniform(-bound, bound, (DOUT, DIN)).astype(np.float32)
    out = kernel(x, gamma, W)
    print("out", out.shape, out.dtype, float(np.abs(out).mean()))


# revision 3
# speedup vs baseline: 1.1741x; 1.0350x over previous
"""BitLinear (RMSNorm + per-token int8 act fake-quant + ternary weight
fake-quant + linear) Trainium2 Bass kernel, data-parallel over 8 NeuronCores.

Strategy
--------
Tokens (B*S = 32768) are sharded 8 ways (4096 tokens/core); W is replicated.
Host prep: ternary weight quantization (per-tensor stat, fp32 semantics
matching the reference) ships ONCE as fp8e4 in k-major layout (32KB/core);
x ships as bf16; the output returns as bf16 and is upcast on host (both
well inside the 2e-2 gate).

The matmul runs in fp8 DoubleRow mode (2 MACs/cell/cycle) while staying
integer-exact via a hi/lo split of the int8 activations:
    q = round(xn*a) in [-127,127]
    h = floor((q+8)/16) in [-8,8];  l = q - 16h in [-8,7]
    q*w = (16h)*w + l*w          (16h in [-128,128] and l fp8e4-exact)
The DoubleRow rhs pair is (w, w) via a stride-0 broadcast AP
(unsqueeze+to_broadcast) so W resides once in SBUF — verified correct on
hardware — and the lhs pair is (16h, l).

Numeric tricks (all ops use only device-codegen-supported shapes; AluOpType
mod / abs_max TT and any GPSIMD scalar_tensor_tensor or PSUM access do NOT
compile for CoreV3 — learned the hard way):
  * q via fp32 magic round: u = x*s + 1.5*2^23 on ACT, then a 2x-mode
    subtract on DVE/Pool -> bf16.
  * quant scale s = 127/(absmax + 1e-5): the rms factor cancels out of
    a_scale*rms, so the sumsq/sqrt/rms chain feeds only the OUTPUT scale
    s3 = (mx+1e-5)*rms/(127*ws), consumed one super-tile later by the
    deferred psum evacuation -> absmax alone gates the quant chain.
  * h via bf16 magic at ulp 16: t2 = qt + 3072.5 -> bf16 rounds (never a
    tie) to 3072 + 16h; Pool subtracts 3072 -> fp8 16h (1-op TS shapes
    only); l = qt - 16h via DVE scalar_tensor_tensor per kt-half (the
    kt-half granularity lets each 8-kt matmul group start while the next
    half's pair is still being built).

Pipeline structure per 256-token super-tile (2 PE tiles, 16 of them/core):
front-end (stats, scale, magic round, transpose, x prefetch) is emitted one
super-tile AHEAD of the back-end (t2, pair build, 128 matmuls, deferred
epilogue of the previous tile) — cfg["skew"]=1 — which repositions the tile
framework's semaphore-recycle barriers so PE runahead survives them. PSUM
is split per (half, og-pair) into 4 x [128,1024] tiles so a start-matmul's
WAR on the previous evacuation releases per quarter. Out-DMA rides the SP
queue (never head-of-line blocks ACT), x prefetches ride SP emitted after
the transpose, weights stream in 16 chunks mostly inside super-tile 0.

Timeline-sim: 271.5us/core (fp8 DoubleRow PE roofline 218.5us; v1 baseline
311.6us). Measured engine busy/ST: ACT ~12.4us, DVE ~10.5, Pool ~11.6,
DMA ~10.1 vs PE cadence 13.66us. Remaining gap: ~20us fill, ~8us tail +
drain, ~15us pstate ramp at the early streak resets, residual settling
stalls. NOTE: slicing a rearranged AP view silently mis-strides on
hardware — slice the underlying tile first, then rearrange.
"""
import numpy as np
from contextlib import ExitStack

import concourse.bacc as bacc
import concourse.tile as tile
from concourse import mybir
from concourse.bass_utils import run_bass_kernel_spmd

F32 = mybir.dt.float32
BF16 = mybir.dt.bfloat16
FP8 = mybir.dt.float8e4
AL = mybir.AluOpType
AF = mybir.ActivationFunctionType
AX = mybir.AxisListType
PM = mybir.MatmulPerfMode.DoubleRow

B, S, DIN, DOUT = 4, 8192, 2048, 2048
NCORES = 8
TOK = B * S                  # 32768
TPC = TOK // NCORES          # 4096 tokens per core
NST = TPC // 256             # 16 super-tiles (2 PE tiles each) per core
KB = DIN // 128              # 16 contraction blocks per token tile
OGW = 512                    # psum free dim per matmul
OG = DOUT // OGW             # 4 output groups

C_MAGIC = 12582912.0         # 1.5 * 2^23: fp32 +C rounds to nearest int (RNE)
C_UNB = C_MAGIC              # unbias to q in [-127,127]
# bf16 magic at ulp=16: y = q + 0.5 + 3072 in [2945.5, 3199.5] rounds (bf16
# ulp 16 in [2048,4096), never a tie) to 3072 + 16*h with h = floor((q+8)/16)
T2_BIAS = 3072.5
T2_SUB = 3072.0

_CACHE = {}


def _build(cfg=None):
    cfg = {**dict(xp=5, up=5, qp=3, qtp=4, hlp=2, op=2, tmp=1,
                  evac="act", ub1=2, skew=1), **(cfg or {})}
    nc = bacc.Bacc("TRN2", target_bir_lowering=False, debug=False,
                   num_devices=NCORES)
    x_d = nc.declare_dram_parameter("x", [TPC, DIN], BF16, isOutput=False)
    w2_d = nc.declare_dram_parameter("w2", [DIN, DOUT], FP8, isOutput=False)
    sc_d = nc.declare_dram_parameter("sc", [1, 1], F32, isOutput=False)
    o_d = nc.declare_dram_parameter("out", [TPC, DOUT], BF16, isOutput=True)

    with tile.TileContext(nc, pool_alloc_mode=cfg.get("pam", "stack")) as tc:
        with ExitStack() as ctx:
            cst = ctx.enter_context(tc.tile_pool(name="cst", bufs=1))
            w2p = ctx.enter_context(tc.tile_pool(name="w2p", bufs=1))
            xp = ctx.enter_context(tc.tile_pool(name="xp", bufs=cfg["xp"]))
            up = ctx.enter_context(tc.tile_pool(name="up", bufs=cfg["up"]))
            qp = ctx.enter_context(tc.tile_pool(name="qp", bufs=cfg["qp"]))
            qtp = ctx.enter_context(tc.tile_pool(name="qtp", bufs=cfg["qtp"]))
            hlp = ctx.enter_context(tc.tile_pool(name="hlp", bufs=cfg["hlp"]))
            op = ctx.enter_context(tc.tile_pool(name="op", bufs=cfg["op"]))
            st = ctx.enter_context(tc.tile_pool(name="st", bufs=6))
            tmp = ctx.enter_context(tc.tile_pool(name="tmp", bufs=cfg["tmp"]))
            pso = ctx.enter_context(tc.tile_pool(name="pso", bufs=1, space="PSUM"))

            # ---- constants; first x super-tile (split per half) first
            xpre = xp.tile([128, 2, DIN], BF16, name="xt", tag="xtile")
            for h in range(2):
                nc.sync.dma_start(
                    out=xpre[:, h, :],
                    in_=x_d[h * 128:(h + 1) * 128, :])
            scb = cst.tile([128, 1], F32, name="scb")
            nc.sync.dma_start(out=scb, in_=sc_d[:].to_broadcast((128, 1)))
            inv127 = scb[:, 0:1]   # 1/(127*w_scale)
            cmag = cst.tile([128, 1], F32, name="cmag")
            nc.vector.memset(cmag, C_MAGIC)
            ceps = cst.tile([128, 1], F32, name="ceps")
            nc.vector.memset(ceps, 1e-6)
            warmt = cst.tile([128, 1], F32, name="warmt")
            nc.scalar.activation(out=warmt, in_=cmag, func=AF.Square)
            nc.scalar.activation(out=warmt, in_=cmag, func=AF.Sqrt)

            # ---- weight pairs (-16w, w), fp8 k-major, [128, kt, 2, DOUT]
            # Only chunks 0-1 load ahead of super-tile 0's q transpose; the
            # rest are emitted inside super-tile 0 (they arrive well before
            # the matmul wave reaches them). x for super-tile 1 also loads
            # up front so its front-end is never starved.
            w2 = w2p.tile([128, KB, DOUT], FP8, name="w2")
            nc.sync.dma_start(out=w2[:, 0, :], in_=w2_d[0:128, :])
            xpre1 = xp.tile([128, 2, DIN], BF16, name="xt", tag="xtile")
            nc.sync.dma_start(
                out=xpre1,
                in_=x_d[256:512, :].rearrange("(h p) k -> p h k", p=128))
            nc.sync.dma_start(out=w2[:, 1, :], in_=w2_d[128:256, :])

            def wrhs(kt, og):
                return w2[:, kt, og * OGW:(og + 1) * OGW] \
                    .unsqueeze(1).to_broadcast((128, 2, OGW))

            pend = []

            def _flush_epilogue(e):
                # GPSIMD cannot access PSUM on HW: evac on ACT, with the
                # last cfg["evd"] og-pair quarters on DVE
                jt, pj, otj, s3j = e
                evd = cfg.get("evd", 0)
                for h in range(2):
                    for g in range(2):
                        dst = otj[:, h, g * 1024:(g + 1) * 1024]
                        if 2 * h + g >= 4 - evd:
                            nc.vector.tensor_scalar(out=dst, in0=pj[2 * h + g],
                                                    scalar1=s3j[:, h:h + 1],
                                                    scalar2=None, op0=AL.mult)
                        else:
                            nc.scalar.mul(out=dst, in_=pj[2 * h + g],
                                          mul=s3j[:, h:h + 1])
                if cfg.get("odh"):
                    for h in range(2):
                        nc.sync.dma_start(
                            out=o_d[jt * 256 + h * 128:
                                    jt * 256 + (h + 1) * 128, :],
                            in_=otj[:, h, :])
                else:
                    nc.sync.dma_start(
                        out=o_d[jt * 256:(jt + 1) * 256, :]
                        .rearrange("(h p) k -> p h k", p=128),
                        in_=otj)

            xtiles = [xpre, xpre1]
            fes = {}
            skew = cfg.get("skew", 0)

            def backend(bit):
                _, bq, bqt, bqhl, bot, bs3 = fes.pop(bit)
                t2 = bq.rearrange("p h (a b) -> p h a b", a=KB)
                nc.vector.tensor_scalar(out=t2, in0=bqt, scalar1=T2_BIAS,
                                        scalar2=None, op0=AL.add)
                poss = []
                for h in range(2):
                    pg = [pso.tile([128, 2 * OGW], F32, name=f"pos{h}{g}",
                                   tag=f"pos{h}{g}") for g in range(2)]
                    poss.extend(pg)
                    for j in range(2):
                        ks = slice(j * HK2, (j + 1) * HK2)
                        t2hj = bq[:, h, j * HK2 * 128:(j + 1) * HK2 * 128] \
                            .rearrange("p (a b) -> p a b", a=HK2)
                        nc.gpsimd.tensor_scalar(out=bqhl[:, h, ks, 0, :],
                                                in0=t2hj, scalar1=T2_SUB,
                                                scalar2=None, op0=AL.subtract)
                        nc.vector.scalar_tensor_tensor(
                            out=bqhl[:, h, ks, 1, :],
                            in0=bqhl[:, h, ks, 0, :], scalar=-1.0,
                            in1=bqt[:, h, ks], op0=AL.mult, op1=AL.add)
                        for kt in range(j * HK2, (j + 1) * HK2):
                            for og in range(OG):
                                nc.tensor.matmul(
                                    pg[og // 2][:, (og % 2) * OGW:
                                                (og % 2 + 1) * OGW],
                                    lhsT=bqhl[:, h, kt, :, :],
                                    rhs=wrhs(kt, og),
                                    start=(kt == 0), stop=(kt == KB - 1),
                                    perf_mode=PM)
                if pend:
                    _flush_epilogue(pend.pop(0))
                pend.append((bit, poss, bot, bs3))

            HK2 = KB // 2
            for it in range(NST):
                xt = xtiles[it]
                q = qp.tile([128, 2, DIN], BF16, name="q")
                qt = qtp.tile([128, 2, KB, 128], BF16, name="qt")
                qhl = hlp.tile([128, 2, KB, 2, 128], FP8, name="qhl")
                ot = op.tile([128, 2, DOUT], BF16, name="ot")
                ss = st.tile([128, 2], F32, name="ss")
                mx = st.tile([128, 2], F32, name="mx")
                s3 = st.tile([128, 2], F32, name="s3", tag="s3")
                HK = KB // 2

                if it == 0:
                    # fine-grained early super-tiles: per-half chain so the
                    # first matmuls fire early and runahead builds fast
                    s3 = st.tile([128, 2], F32, name="s3")
                    poss = []
                    scrs = []
                    for h in range(2):
                        scr_h = up.tile([128, DIN], F32, name="scr")
                        scrs.append(scr_h)
                        nc.scalar.activation(out=scr_h, in_=xt[:, h, :],
                                             func=AF.Square,
                                             accum_out=ss[:, h:h + 1])
                        nc.vector.tensor_reduce(out=mx[:, h:h + 1],
                                                in_=xt[:, h, :],
                                                axis=AX.X, op=AL.max,
                                                apply_absolute_value=True)
                        ts1 = st.tile([128, 1], F32, name="ts1",
                                      tag=f"ts1{h}")
                        nc.vector.tensor_scalar(out=ts1, in0=mx[:, h:h + 1],
                                                scalar1=1e-5,
                                                scalar2=1.0 / 127.0,
                                                op0=AL.add, op1=AL.mult)
                        s = st.tile([128, 1], F32, name="s", tag=f"s{h}")
                        nc.vector.reciprocal(out=s, in_=ts1)
                        sqv = st.tile([128, 1], F32, name="sqv", tag=f"sqv{h}")
                        nc.scalar.activation(out=sqv, in_=ss[:, h:h + 1],
                                             func=AF.Sqrt, bias=ceps,
                                             scale=1.0 / DIN)
                        rms = st.tile([128, 1], F32, name="rms", tag=f"rms{h}")
                        nc.vector.reciprocal(out=rms, in_=sqv)
                        nc.vector.scalar_tensor_tensor(out=s3[:, h:h + 1],
                                                       in0=rms, scalar=127.0,
                                                       in1=ts1, op0=AL.mult,
                                                       op1=AL.mult)
                        nc.vector.tensor_scalar(out=s3[:, h:h + 1],
                                                in0=s3[:, h:h + 1],
                                                scalar1=inv127, scalar2=None,
                                                op0=AL.mult)
                        nc.scalar.activation(out=scr_h, in_=xt[:, h, :],
                                             func=AF.Identity, bias=cmag,
                                             scale=s[:, 0:1])
                        nc.vector.tensor_scalar(out=q[:, h, :],
                                                in0=scr_h,
                                                scalar1=C_UNB, scalar2=None,
                                                op0=AL.subtract)
                        nc.sync.dma_start_transpose(qt[:, h], q[:, h, :])
                        if it == 0 and h == 0:
                            for kt in range(2, KB):
                                nc.sync.dma_start(
                                    out=w2[:, kt, :],
                                    in_=w2_d[kt * 128:(kt + 1) * 128, :])
                        t2 = q[:, h, :].rearrange("p (a b) -> p a b", a=KB)
                        nc.vector.tensor_scalar(out=t2, in0=qt[:, h],
                                                scalar1=T2_BIAS, scalar2=None,
                                                op0=AL.add)
                        pg = [pso.tile([128, 2 * OGW], F32,
                                       name=f"pos{h}{g}", tag=f"pos{h}{g}")
                              for g in range(2)]
                        poss.extend(pg)
                        for j in range(2):
                            ks = slice(j * HK, (j + 1) * HK)
                            t2hj = q[:, h, j * HK * 128:(j + 1) * HK * 128] \
                                .rearrange("p (a b) -> p a b", a=HK)
                            nc.gpsimd.tensor_scalar(out=qhl[:, h, ks, 0, :],
                                                    in0=t2hj, scalar1=T2_SUB,
                                                    scalar2=None,
                                                    op0=AL.subtract)
                            nc.vector.scalar_tensor_tensor(
                                out=qhl[:, h, ks, 1, :],
                                in0=qhl[:, h, ks, 0, :], scalar=-1.0,
                                in1=qt[:, h, ks], op0=AL.mult, op1=AL.add)
                            for kt in range(j * HK, (j + 1) * HK):
                                for og in range(OG):
                                    nc.tensor.matmul(
                                        pg[og // 2][:, (og % 2) * OGW:
                                                    (og % 2 + 1) * OGW],
                                        lhsT=qhl[:, h, kt, :, :],
                                        rhs=wrhs(kt, og),
                                        start=(kt == 0), stop=(kt == KB - 1),
                                        perf_mode=PM)
                    # prefetch x two super-tiles ahead (0 and 1 preloaded)
                    if it + 2 < NST:
                        xn = xp.tile([128, 2, DIN], BF16, name="xt",
                                     tag="xtile")
                        nc.sync.dma_start(
                            out=xn,
                            in_=x_d[(it + 2) * 256:(it + 3) * 256, :]
                            .rearrange("(h p) k -> p h k", p=128))
                        xtiles.append(xn)
                    if pend:
                        _flush_epilogue(pend.pop(0))
                    pend.append((it, poss, ot, s3))
                    continue

                # ---- steady state: split into front-end / back-end so
                # back-end(it) can be emitted cfg["skew"] super-tiles behind
                fes[it] = (xt, q, qt, qhl, ot, s3)
                scrs = []
                for h in range(2):
                    scr_h = up.tile([128, DIN], F32, name="scr")
                    scrs.append(scr_h)
                    nc.scalar.activation(out=scr_h, in_=xt[:, h, :],
                                         func=AF.Square,
                                         accum_out=ss[:, h:h + 1])
                for h in range(2):
                    nc.vector.tensor_reduce(out=mx[:, h:h + 1],
                                            in_=xt[:, h, :], axis=AX.X,
                                            op=AL.max,
                                            apply_absolute_value=True)
                # quant scale: s = 127/(mx + 1e-5) — the rms cancels out of
                # a_scale*rms, so the sumsq/sqrt path is NOT on the q chain
                ts1 = st.tile([128, 2], F32, name="ts1", tag="ts1")
                nc.vector.tensor_scalar(out=ts1, in0=mx, scalar1=1e-5,
                                        scalar2=1.0 / 127.0, op0=AL.add,
                                        op1=AL.mult)
                s = st.tile([128, 2], F32, name="s2", tag="s2")
                nc.vector.reciprocal(out=s, in_=ts1)
                # output scale s3 = (mx+1e-5)*rms/(127*ws) — off-chain, only
                # needed by the (deferred) psum evacuation
                sqv = st.tile([128, 2], F32, name="sqv2", tag="sqv2")
                nc.scalar.activation(out=sqv, in_=ss, func=AF.Sqrt, bias=ceps,
                                     scale=1.0 / DIN)
                rms = st.tile([128, 2], F32, name="rms2", tag="rms2")
                nc.vector.reciprocal(out=rms, in_=sqv)
                nc.vector.scalar_tensor_tensor(out=s3, in0=rms,
                                               scalar=127.0, in1=ts1,
                                               op0=AL.mult, op1=AL.mult)
                nc.vector.tensor_scalar(out=s3, in0=s3, scalar1=inv127,
                                        scalar2=None, op0=AL.mult)

                # qs = RNE(x*s) + 8 via fp32 magic (per-half scratch)
                for h in range(2):
                    nc.scalar.activation(out=scrs[h], in_=xt[:, h, :],
                                         func=AF.Identity, bias=cmag,
                                         scale=s[:, h:h + 1])
                    ub = cfg.get("ub1")
                    eng = nc.gpsimd if (ub == 2 or (h == 1 and ub)) \
                        else nc.vector
                    eng.tensor_scalar(out=q[:, h, :], in0=scrs[h],
                                      scalar1=C_UNB, scalar2=None,
                                      op0=AL.subtract)
                nc.sync.dma_start_transpose(
                    qt, q.rearrange("p h k -> p (h k)"))
                # prefetch x two super-tiles ahead (xp buffers keep the
                # slot-free wait at zero; emitted after the transpose so it
                # can never delay it in the queue)
                if it + 2 < NST:
                    xn = xp.tile([128, 2, DIN], BF16, name="xt", tag="xtile")
                    nc.sync.dma_start(
                        out=xn,
                        in_=x_d[(it + 2) * 256:(it + 3) * 256, :]
                        .rearrange("(h p) k -> p h k", p=128))
                    xtiles.append(xn)

                bi = it - skew
                if bi >= 1:
                    backend(bi)
                if it == NST - 1:
                    for bi in range(max(1, NST - skew), NST):
                        backend(bi)
                    jt, pj, otj, s3j = pend.pop()
                    for h in range(2):
                        for g in range(2):
                            nc.scalar.mul(
                                out=otj[:, h, g * 1024:(g + 1) * 1024],
                                in_=pj[2 * h + g], mul=s3j[:, h:h + 1])
                        nc.sync.dma_start(
                            out=o_d[jt * 256 + h * 128:
                                    jt * 256 + (h + 1) * 128, :],
                            in_=otj[:, h, :])

    nc.compile()
    return nc


def kernel(x, gamma, W):
    import ml_dtypes

    x = np.asarray(x, dtype=np.float32)
    gamma = np.asarray(gamma, dtype=np.float32)
    W = np.asarray(W, dtype=np.float32)

    # host prep: ternary weight pairs + the global scale, fp32 semantics
    # matching the reference: w_scale = 1/(mean|W| + 1e-5)
    m = np.float32(np.abs(W).astype(np.float64).mean())
    denom = np.float32(m + np.float32(1e-5))
    ws = np.float32(np.float32(1.0) / denom)
    wqh = np.clip(np.rint((W * ws).astype(np.float32)), -1.0, 1.0)
    w2 = np.ascontiguousarray(wqh.T).astype(ml_dtypes.float8_e4m3)
    sc = np.array([[1.0 / (127.0 * float(ws))]], dtype=np.float32)

    if not np.all(gamma == 1.0):
        x = x * gamma  # reference order is (x*rms)*gamma; ~1ulp difference
    xb = x.reshape(TOK, DIN).astype(ml_dtypes.bfloat16)

    if "nc" not in _CACHE:
        _CACHE["nc"] = _build()
    nc = _CACHE["nc"]

    in_maps = [
        {"x": xb[c * TPC:(c + 1) * TPC], "w2": w2, "sc": sc}
        for c in range(NCORES)
    ]
    res = run_bass_kernel_spmd(nc, in_maps, list(range(NCORES)))
    out = np.concatenate([res.results[c]["out"] for c in range(NCORES)],
                         axis=0)
    return out.astype(np.float32).reshape(B, S, DOUT)


if __name__ == "__main__":
    rng = np.random.default_rng(0)
    x = rng.standard_normal((B, S, DIN), dtype=np.float32)
    gamma = np.ones((DIN,), dtype=np.float32)
    bound = 1.0 / np.sqrt(DIN)
    W = rng.uniform(-bound, bound, (DOUT, DIN)).astype(np.float32)
    out = kernel(x, gamma, W)
    print("out", out.shape, out.dtype, float(np.abs(out).mean()))


# revision 4
# speedup vs baseline: 1.1787x; 1.0039x over previous
"""BitLinear (RMSNorm + per-token int8 act fake-quant + ternary weight
fake-quant + linear) Trainium2 Bass kernel, data-parallel over 8 NeuronCores.

Strategy
--------
Tokens (B*S = 32768) are sharded 8 ways (4096 tokens/core); W is replicated.
Host prep: ternary weight quantization (per-tensor stat, fp32 semantics
matching the reference) ships ONCE as fp8e4 in k-major layout (32KB/core);
x ships as bf16; the output returns as bf16 and is upcast on host (both
well inside the 2e-2 gate).

The matmul runs in fp8 DoubleRow mode (2 MACs/cell/cycle) while staying
integer-exact via a hi/lo split of the int8 activations:
    q = round(xn*a) in [-127,127]
    h = floor((q+8)/16) in [-8,8];  l = q - 16h in [-8,7]
    q*w = (16h)*w + l*w          (16h in [-128,128] and l fp8e4-exact)
The DoubleRow rhs pair is (w, w) via a stride-0 broadcast AP
(unsqueeze+to_broadcast) so W resides once in SBUF — verified correct on
hardware — and the lhs pair is (16h, l).

Numeric tricks (all ops use only device-codegen-supported shapes; AluOpType
mod / abs_max TT and any GPSIMD scalar_tensor_tensor or PSUM access do NOT
compile for CoreV3 — learned the hard way):
  * q via fp32 magic round: u = x*s + 1.5*2^23 on ACT, then a 2x-mode
    subtract on DVE/Pool -> bf16.
  * quant scale s = 127/(absmax + 1e-5): the rms factor cancels out of
    a_scale*rms, so the sumsq/sqrt/rms chain feeds only the OUTPUT scale
    s3 = (mx+1e-5)*rms/(127*ws), consumed one super-tile later by the
    deferred psum evacuation -> absmax alone gates the quant chain.
  * h via bf16 magic at ulp 16: t2 = qt + 3072.5 -> bf16 rounds (never a
    tie) to 3072 + 16h; Pool subtracts 3072 -> fp8 16h (1-op TS shapes
    only); l = qt - 16h via DVE scalar_tensor_tensor per kt-half (the
    kt-half granularity lets each 8-kt matmul group start while the next
    half's pair is still being built).

Pipeline structure per 256-token super-tile (2 PE tiles, 16 of them/core):
front-end (stats, scale, magic round, transpose, x prefetch) is emitted one
super-tile AHEAD of the back-end (t2, pair build, 128 matmuls, deferred
epilogue of the previous tile) — cfg["skew"]=1 — which repositions the tile
framework's semaphore-recycle barriers so PE runahead survives them. PSUM
is split per (half, og-pair) into 4 x [128,1024] tiles so a start-matmul's
WAR on the previous evacuation releases per quarter. Out-DMA rides the SP
queue (never head-of-line blocks ACT), x prefetches ride SP emitted after
the transpose, weights stream in 16 chunks mostly inside super-tile 0.

Timeline-sim: 270.4us/core (fp8 DoubleRow PE roofline 218.5us; v1 baseline
311.6us). Measured engine busy/ST: ACT ~12.4us, DVE ~10.5, Pool ~11.6,
DMA ~10.1 vs PE cadence 13.66us. Remaining gap: ~20us fill, ~8us tail +
drain, ~15us pstate ramp at the early streak resets, residual settling
stalls. NOTE: slicing a rearranged AP view silently mis-strides on
hardware — slice the underlying tile first, then rearrange.
"""
import numpy as np
from contextlib import ExitStack

import concourse.bacc as bacc
import concourse.tile as tile
from concourse import mybir
from concourse.bass_utils import run_bass_kernel_spmd

F32 = mybir.dt.float32
BF16 = mybir.dt.bfloat16
FP8 = mybir.dt.float8e4
AL = mybir.AluOpType
AF = mybir.ActivationFunctionType
AX = mybir.AxisListType
PM = mybir.MatmulPerfMode.DoubleRow

B, S, DIN, DOUT = 4, 8192, 2048, 2048
NCORES = 8
TOK = B * S                  # 32768
TPC = TOK // NCORES          # 4096 tokens per core
NST = TPC // 256             # 16 super-tiles (2 PE tiles each) per core
KB = DIN // 128              # 16 contraction blocks per token tile
OGW = 512                    # psum free dim per matmul
OG = DOUT // OGW             # 4 output groups

C_MAGIC = 12582912.0         # 1.5 * 2^23: fp32 +C rounds to nearest int (RNE)
C_UNB = C_MAGIC              # unbias to q in [-127,127]
# bf16 magic at ulp=16: y = q + 0.5 + 3072 in [2945.5, 3199.5] rounds (bf16
# ulp 16 in [2048,4096), never a tie) to 3072 + 16*h with h = floor((q+8)/16)
T2_BIAS = 3072.5
T2_SUB = 3072.0

_CACHE = {}


def _build(cfg=None):
    cfg = {**dict(xp=5, up=5, qp=3, qtp=4, hlp=2, op=2, tmp=1,
                  evac="act", ub1=2, skew=1), **(cfg or {})}
    nc = bacc.Bacc("TRN2", target_bir_lowering=False, debug=False,
                   num_devices=NCORES)
    x_d = nc.declare_dram_parameter("x", [TPC, DIN], BF16, isOutput=False)
    w2_d = nc.declare_dram_parameter("w2", [DIN, DOUT], FP8, isOutput=False)
    sc_d = nc.declare_dram_parameter("sc", [1, 1], F32, isOutput=False)
    o_d = nc.declare_dram_parameter("out", [TPC, DOUT], BF16, isOutput=True)

    with tile.TileContext(nc, pool_alloc_mode=cfg.get("pam", "stack")) as tc:
        with ExitStack() as ctx:
            cst = ctx.enter_context(tc.tile_pool(name="cst", bufs=1))
            w2p = ctx.enter_context(tc.tile_pool(name="w2p", bufs=1))
            xp = ctx.enter_context(tc.tile_pool(name="xp", bufs=cfg["xp"]))
            up = ctx.enter_context(tc.tile_pool(name="up", bufs=cfg["up"]))
            qp = ctx.enter_context(tc.tile_pool(name="qp", bufs=cfg["qp"]))
            qtp = ctx.enter_context(tc.tile_pool(name="qtp", bufs=cfg["qtp"]))
            hlp = ctx.enter_context(tc.tile_pool(name="hlp", bufs=cfg["hlp"]))
            op = ctx.enter_context(tc.tile_pool(name="op", bufs=cfg["op"]))
            st = ctx.enter_context(tc.tile_pool(name="st", bufs=6))
            tmp = ctx.enter_context(tc.tile_pool(name="tmp", bufs=cfg["tmp"]))
            pso = ctx.enter_context(tc.tile_pool(name="pso", bufs=1, space="PSUM"))

            # ---- constants; first x super-tile (split per half) first
            xpre = xp.tile([128, 2, DIN], BF16, name="xt", tag="xtile")
            for h in range(2):
                nc.sync.dma_start(
                    out=xpre[:, h, :],
                    in_=x_d[h * 128:(h + 1) * 128, :])
            scb = cst.tile([128, 1], F32, name="scb")
            nc.sync.dma_start(out=scb, in_=sc_d[:].to_broadcast((128, 1)))
            inv127 = scb[:, 0:1]   # 1/(127*w_scale)
            cmag = cst.tile([128, 1], F32, name="cmag")
            nc.vector.memset(cmag, C_MAGIC)
            ceps = cst.tile([128, 1], F32, name="ceps")
            nc.vector.memset(ceps, 1e-6)
            warmt = cst.tile([128, 1], F32, name="warmt")
            nc.scalar.activation(out=warmt, in_=cmag, func=AF.Square)
            nc.scalar.activation(out=warmt, in_=cmag, func=AF.Sqrt)

            # ---- weight pairs (-16w, w), fp8 k-major, [128, kt, 2, DOUT]
            # Only chunks 0-1 load ahead of super-tile 0's q transpose; the
            # rest are emitted inside super-tile 0 (they arrive well before
            # the matmul wave reaches them). x for super-tile 1 also loads
            # up front so its front-end is never starved.
            w2 = w2p.tile([128, KB, DOUT], FP8, name="w2")
            nc.sync.dma_start(out=w2[:, 0, :], in_=w2_d[0:128, :])
            xpre1 = xp.tile([128, 2, DIN], BF16, name="xt", tag="xtile")
            nc.sync.dma_start(
                out=xpre1,
                in_=x_d[256:512, :].rearrange("(h p) k -> p h k", p=128))
            nc.sync.dma_start(out=w2[:, 1, :], in_=w2_d[128:256, :])

            def wrhs(kt, og):
                return w2[:, kt, og * OGW:(og + 1) * OGW] \
                    .unsqueeze(1).to_broadcast((128, 2, OGW))

            # PE warm-up train: dummy matmuls (weights x weights into a
            # psum tile that the first real start=True group overwrites)
            # keep the PE p-state ramped through the pipeline-fill window so
            # the real matmul wave starts at full clock
            nwarm = cfg.get("warm", 0)
            if nwarm:
                wpos = pso.tile([128, 2 * OGW], F32, name="pos00",
                                tag="pos00")
                for i in range(nwarm):
                    nc.tensor.matmul(wpos[:, 0:OGW], lhsT=w2[:, 0, 0:128],
                                     rhs=w2[:, 0, 0:OGW], start=True,
                                     stop=True)

            pend = []

            def _flush_epilogue(e):
                # GPSIMD cannot access PSUM on HW: evac on ACT, with the
                # last cfg["evd"] og-pair quarters on DVE
                jt, pj, otj, s3j = e
                evd = cfg.get("evd", 0)
                for h in range(2):
                    for g in range(2):
                        dst = otj[:, h, g * 1024:(g + 1) * 1024]
                        if 2 * h + g >= 4 - evd:
                            nc.vector.tensor_scalar(out=dst, in0=pj[2 * h + g],
                                                    scalar1=s3j[:, h:h + 1],
                                                    scalar2=None, op0=AL.mult)
                        else:
                            nc.scalar.mul(out=dst, in_=pj[2 * h + g],
                                          mul=s3j[:, h:h + 1])
                if cfg.get("odh"):
                    for h in range(2):
                        nc.sync.dma_start(
                            out=o_d[jt * 256 + h * 128:
                                    jt * 256 + (h + 1) * 128, :],
                            in_=otj[:, h, :])
                else:
                    nc.sync.dma_start(
                        out=o_d[jt * 256:(jt + 1) * 256, :]
                        .rearrange("(h p) k -> p h k", p=128),
                        in_=otj)

            xtiles = [xpre, xpre1]
            fes = {}
            skew = cfg.get("skew", 0)

            def backend(bit):
                _, bq, bqt, bqhl, bot, bs3 = fes.pop(bit)
                t2 = bq.rearrange("p h (a b) -> p h a b", a=KB)
                nc.vector.tensor_scalar(out=t2, in0=bqt, scalar1=T2_BIAS,
                                        scalar2=None, op0=AL.add)
                poss = []
                for h in range(2):
                    pg = [pso.tile([128, 2 * OGW], F32, name=f"pos{h}{g}",
                                   tag=f"pos{h}{g}") for g in range(2)]
                    poss.extend(pg)
                    for j in range(2):
                        ks = slice(j * HK2, (j + 1) * HK2)
                        t2hj = bq[:, h, j * HK2 * 128:(j + 1) * HK2 * 128] \
                            .rearrange("p (a b) -> p a b", a=HK2)
                        nc.gpsimd.tensor_scalar(out=bqhl[:, h, ks, 0, :],
                                                in0=t2hj, scalar1=T2_SUB,
                                                scalar2=None, op0=AL.subtract)
                        nc.vector.scalar_tensor_tensor(
                            out=bqhl[:, h, ks, 1, :],
                            in0=bqhl[:, h, ks, 0, :], scalar=-1.0,
                            in1=bqt[:, h, ks], op0=AL.mult, op1=AL.add)
                        for kt in range(j * HK2, (j + 1) * HK2):
                            for og in range(OG):
                                nc.tensor.matmul(
                                    pg[og // 2][:, (og % 2) * OGW:
                                                (og % 2 + 1) * OGW],
                                    lhsT=bqhl[:, h, kt, :, :],
                                    rhs=wrhs(kt, og),
                                    start=(kt == 0), stop=(kt == KB - 1),
                                    perf_mode=PM)
                if pend:
                    _flush_epilogue(pend.pop(0))
                pend.append((bit, poss, bot, bs3))

            HK2 = KB // 2
            for it in range(NST):
                xt = xtiles[it]
                q = qp.tile([128, 2, DIN], BF16, name="q")
                qt = qtp.tile([128, 2, KB, 128], BF16, name="qt")
                qhl = hlp.tile([128, 2, KB, 2, 128], FP8, name="qhl")
                ot = op.tile([128, 2, DOUT], BF16, name="ot")
                ss = st.tile([128, 2], F32, name="ss")
                mx = st.tile([128, 2], F32, name="mx")
                s3 = st.tile([128, 2], F32, name="s3", tag="s3")
                HK = KB // 2

                if it == 0:
                    # fine-grained early super-tiles: per-half chain so the
                    # first matmuls fire early and runahead builds fast
                    s3 = st.tile([128, 2], F32, name="s3")
                    poss = []
                    scrs = []
                    for h in range(2):
                        scr_h = up.tile([128, DIN], F32, name="scr")
                        scrs.append(scr_h)
                        nc.scalar.activation(out=scr_h, in_=xt[:, h, :],
                                             func=AF.Square,
                                             accum_out=ss[:, h:h + 1])
                        nc.vector.tensor_reduce(out=mx[:, h:h + 1],
                                                in_=xt[:, h, :],
                                                axis=AX.X, op=AL.max,
                                                apply_absolute_value=True)
                        ts1 = st.tile([128, 1], F32, name="ts1",
                                      tag=f"ts1{h}")
                        nc.vector.tensor_scalar(out=ts1, in0=mx[:, h:h + 1],
                                                scalar1=1e-5,
                                                scalar2=1.0 / 127.0,
                                                op0=AL.add, op1=AL.mult)
                        s = st.tile([128, 1], F32, name="s", tag=f"s{h}")
                        nc.vector.reciprocal(out=s, in_=ts1)
                        nc.scalar.activation(out=scr_h, in_=xt[:, h, :],
                                             func=AF.Identity, bias=cmag,
                                             scale=s[:, 0:1])
                        nc.vector.tensor_scalar(out=q[:, h, :],
                                                in0=scr_h,
                                                scalar1=C_UNB, scalar2=None,
                                                op0=AL.subtract)
                        nc.sync.dma_start_transpose(qt[:, h], q[:, h, :])
                        if it == 0 and h == 0:
                            for kt in range(2, KB):
                                nc.sync.dma_start(
                                    out=w2[:, kt, :],
                                    in_=w2_d[kt * 128:(kt + 1) * 128, :])
                        # off-chain output-scale block, deferred past the
                        # transpose so it never sits ahead of the quant chain
                        # in the in-order engine queues
                        sqv = st.tile([128, 1], F32, name="sqv", tag=f"sqv{h}")
                        nc.scalar.activation(out=sqv, in_=ss[:, h:h + 1],
                                             func=AF.Sqrt, bias=ceps,
                                             scale=1.0 / DIN)
                        rms = st.tile([128, 1], F32, name="rms", tag=f"rms{h}")
                        nc.vector.reciprocal(out=rms, in_=sqv)
                        nc.vector.scalar_tensor_tensor(out=s3[:, h:h + 1],
                                                       in0=rms, scalar=127.0,
                                                       in1=ts1, op0=AL.mult,
                                                       op1=AL.mult)
                        nc.vector.tensor_scalar(out=s3[:, h:h + 1],
                                                in0=s3[:, h:h + 1],
                                                scalar1=inv127, scalar2=None,
                                                op0=AL.mult)
                        t2 = q[:, h, :].rearrange("p (a b) -> p a b", a=KB)
                        nc.vector.tensor_scalar(out=t2, in0=qt[:, h],
                                                scalar1=T2_BIAS, scalar2=None,
                                                op0=AL.add)
                        pg = [pso.tile([128, 2 * OGW], F32,
                                       name=f"pos{h}{g}", tag=f"pos{h}{g}")
                              for g in range(2)]
                        poss.extend(pg)
                        for j in range(2):
                            ks = slice(j * HK, (j + 1) * HK)
                            t2hj = q[:, h, j * HK * 128:(j + 1) * HK * 128] \
                                .rearrange("p (a b) -> p a b", a=HK)
                            heng = nc.vector if (h == 0 and
                                                 cfg.get("st0hdve")) \
                                else nc.gpsimd
                            heng.tensor_scalar(out=qhl[:, h, ks, 0, :],
                                               in0=t2hj, scalar1=T2_SUB,
                                               scalar2=None,
                                               op0=AL.subtract)
                            nc.vector.scalar_tensor_tensor(
                                out=qhl[:, h, ks, 1, :],
                                in0=qhl[:, h, ks, 0, :], scalar=-1.0,
                                in1=qt[:, h, ks], op0=AL.mult, op1=AL.add)
                            for kt in range(j * HK, (j + 1) * HK):
                                for og in range(OG):
                                    nc.tensor.matmul(
                                        pg[og // 2][:, (og % 2) * OGW:
                                                    (og % 2 + 1) * OGW],
                                        lhsT=qhl[:, h, kt, :, :],
                                        rhs=wrhs(kt, og),
                                        start=(kt == 0), stop=(kt == KB - 1),
                                        perf_mode=PM)
                    # prefetch x two super-tiles ahead (0 and 1 preloaded)
                    if it + 2 < NST:
                        xn = xp.tile([128, 2, DIN], BF16, name="xt",
                                     tag="xtile")
                        nc.sync.dma_start(
                            out=xn,
                            in_=x_d[(it + 2) * 256:(it + 3) * 256, :]
                            .rearrange("(h p) k -> p h k", p=128))
                        xtiles.append(xn)
                    if pend:
                        _flush_epilogue(pend.pop(0))
                    pend.append((it, poss, ot, s3))
                    continue

                # ---- steady state: split into front-end / back-end so
                # back-end(it) can be emitted cfg["skew"] super-tiles behind
                fes[it] = (xt, q, qt, qhl, ot, s3)
                scrs = []
                for h in range(2):
                    scr_h = up.tile([128, DIN], F32, name="scr")
                    scrs.append(scr_h)
                    nc.scalar.activation(out=scr_h, in_=xt[:, h, :],
                                         func=AF.Square,
                                         accum_out=ss[:, h:h + 1])
                for h in range(2):
                    nc.vector.tensor_reduce(out=mx[:, h:h + 1],
                                            in_=xt[:, h, :], axis=AX.X,
                                            op=AL.max,
                                            apply_absolute_value=True)
                # quant scale: s = 127/(mx + 1e-5) — the rms cancels out of
                # a_scale*rms, so the sumsq/sqrt path is NOT on the q chain
                ts1 = st.tile([128, 2], F32, name="ts1", tag="ts1")
                nc.vector.tensor_scalar(out=ts1, in0=mx, scalar1=1e-5,
                                        scalar2=1.0 / 127.0, op0=AL.add,
                                        op1=AL.mult)
                s = st.tile([128, 2], F32, name="s2", tag="s2")
                nc.vector.reciprocal(out=s, in_=ts1)
                # output scale s3 = (mx+1e-5)*rms/(127*ws) — off-chain, only
                # needed by the (deferred) psum evacuation
                sqv = st.tile([128, 2], F32, name="sqv2", tag="sqv2")
                nc.scalar.activation(out=sqv, in_=ss, func=AF.Sqrt, bias=ceps,
                                     scale=1.0 / DIN)
                rms = st.tile([128, 2], F32, name="rms2", tag="rms2")
                nc.vector.reciprocal(out=rms, in_=sqv)
                nc.vector.scalar_tensor_tensor(out=s3, in0=rms,
                                               scalar=127.0, in1=ts1,
                                               op0=AL.mult, op1=AL.mult)
                nc.vector.tensor_scalar(out=s3, in0=s3, scalar1=inv127,
                                        scalar2=None, op0=AL.mult)

                # qs = RNE(x*s) + 8 via fp32 magic (per-half scratch)
                for h in range(2):
                    nc.scalar.activation(out=scrs[h], in_=xt[:, h, :],
                                         func=AF.Identity, bias=cmag,
                                         scale=s[:, h:h + 1])
                    ub = cfg.get("ub1")
                    eng = nc.gpsimd if (ub == 2 or (h == 1 and ub)) \
                        else nc.vector
                    eng.tensor_scalar(out=q[:, h, :], in0=scrs[h],
                                      scalar1=C_UNB, scalar2=None,
                                      op0=AL.subtract)
                nc.sync.dma_start_transpose(
                    qt, q.rearrange("p h k -> p (h k)"))
                # prefetch x two super-tiles ahead (xp buffers keep the
                # slot-free wait at zero; emitted after the transpose so it
                # can never delay it in the queue)
                if it + 2 < NST:
                    xn = xp.tile([128, 2, DIN], BF16, name="xt", tag="xtile")
                    nc.sync.dma_start(
                        out=xn,
                        in_=x_d[(it + 2) * 256:(it + 3) * 256, :]
                        .rearrange("(h p) k -> p h k", p=128))
                    xtiles.append(xn)

                bi = it - skew
                if bi >= 1:
                    backend(bi)
                if it == NST - 1:
                    for bi in range(max(1, NST - skew), NST):
                        backend(bi)
                    # final epilogue: spread the two quarters of each half
                    # across ACT and DVE and write back per quarter so only
                    # one 1038ns evac + one small DMA are ever exposed
                    jt, pj, otj, s3j = pend.pop()
                    for h in range(2):
                        for g in range(2):
                            dst = otj[:, h, g * 1024:(g + 1) * 1024]
                            if g == 0:
                                nc.vector.tensor_scalar(
                                    out=dst, in0=pj[2 * h + g],
                                    scalar1=s3j[:, h:h + 1], scalar2=None,
                                    op0=AL.mult)
                            else:
                                nc.scalar.mul(out=dst, in_=pj[2 * h + g],
                                              mul=s3j[:, h:h + 1])
                            nc.sync.dma_start(
                                out=o_d[jt * 256 + h * 128:
                                        jt * 256 + (h + 1) * 128,
                                        g * 1024:(g + 1) * 1024],
                                in_=dst)

    nc.compile()
    return nc


def kernel(x, gamma, W):
    import ml_dtypes

    x = np.asarray(x, dtype=np.float32)
    gamma = np.asarray(gamma, dtype=np.float32)
    W = np.asarray(W, dtype=np.float32)

    # host prep: ternary weight pairs + the global scale, fp32 semantics
    # matching the reference: w_scale = 1/(mean|W| + 1e-5)
    m = np.float32(np.abs(W).astype(np.float64).mean())
    denom = np.float32(m + np.float32(1e-5))
    ws = np.float32(np.float32(1.0) / denom)
    wqh = np.clip(np.rint((W * ws).astype(np.float32)), -1.0, 1.0)
    w2 = np.ascontiguousarray(wqh.T).astype(ml_dtypes.float8_e4m3)
    sc = np.array([[1.0 / (127.0 * float(ws))]], dtype=np.float32)

    if not np.all(gamma == 1.0):
        x = x * gamma  # reference order is (x*rms)*gamma; ~1ulp difference
    xb = x.reshape(TOK, DIN).astype(ml_dtypes.bfloat16)

    if "nc" not in _CACHE:
        _CACHE["nc"] = _build()
    nc = _CACHE["nc"]

    in_maps = [
        {"x": xb[c * TPC:(c + 1) * TPC], "w2": w2, "sc": sc}
        for c in range(NCORES)
    ]
    res = run_bass_kernel_spmd(nc, in_maps, list(range(NCORES)))
    out = np.concatenate([res.results[c]["out"] for c in range(NCORES)],
                         axis=0)
    return out.astype(np.float32).reshape(B, S, DOUT)


if __name__ == "__main__":
    rng = np.random.default_rng(0)
    x = rng.standard_normal((B, S, DIN), dtype=np.float32)
    gamma = np.ones((DIN,), dtype=np.float32)
    bound = 1.0 / np.sqrt(DIN)
    W = rng.uniform(-bound, bound, (DOUT, DIN)).astype(np.float32)
    out = kernel(x, gamma, W)
    print("out", out.shape, out.dtype, float(np.abs(out).mean()))
